# revision 48
# baseline (speedup 1.0000x reference)
"""Trainium2 Bass kernel for nn_AttentionLayer (B=4, C=256, N=4096, CR=32).

Sharding: 8 cores = (batch b in 0..3) x (query-half ih in 0..1).
Each core receives x[b] rotated so its own query half sits at columns
0..2047 (softmax is invariant to key order, so the rotation is exact);
it computes the unnormalized attention output for its 2048 queries and
the softmax denominator; the host divides, adds bias + residual, and
reassembles.

v2 scheme (vs the all-f32r baseline):
  - mm1 (scores) stays f32r, but is augmented with TWO extra contraction
    rows: [f/64; 1/64; m~/64] x [g; gbv; -1] so PSUM holds
    t = (s - m~_i)/64 where m~_i = a*||f_i||^2 + b is a per-query
    estimate of the row max (a,b fitted; exact correctness does not
    depend on the estimate -- it only centers exp into fp8 range).
  - exp via ACT Sigmoid(64*t) ~= exp(s - m~) for the dominant
    (deep-negative) regime, saturating at 1 -> can never overflow fp8.
    Output written directly as fp8e5. A slice of stages instead runs a
    two-instruction custom DVE chain (deg-3 poly q ~= e^u then q^64 with
    a clamp at 1) to offload the saturated ACT engine.
  - mm2 (attention-weighted values) in fp8 with perf_mode=DoubleRow:
    two j-tiles (256 keys) contracted per matmul at 0.5 cycles/col
    (4x faster than f32r). lhsT = [h^T | 1] in fp8e4, rhs = exp tiles
    in fp8e5. The ones row accumulates the denominator.
  - the device returns unnormalized num (32 rows) + den (1 row); the
    softmax division commutes with the 1x1 output conv, so the host
    computes gamma*Wo @ (num/den) + gamma*(Wo@bq + bo) + x.
"""

import numpy as np

B, C, N = 4, 256, 4096
CR = 32
NH = N // 2          # queries per core
G = 512              # i-group width
NCORES = 8

# m~ estimator: m~ = MA * ||f_i||^2 + (MB + delta - bv@bk)  (fitted on the
# score-max distribution; delta shifts tops a few e-folds below 1 so the
# sigmoid cap engages rarely and fp8e5 retains relative precision)
MA = 0.34142
MB = 10.226
MDELTA = 4.0

_CACHE = {}

# custom DVE fast-exp chain: q = deg-3 poly of u=(s-m~)/64 on [-1.25, .25],
# then q^64 with a clamp at 1.0 (consistent with the sigmoid cap).
PC1, PC2, PC3 = 1.00110998, 0.50122937, 0.14628461
CLO = -1.25


def _register_dve_ops():
    import numpy as np
    import concourse.dve_ops as dve_ops
    if hasattr(dve_ops, "_ANT_EXP_OPS"):
        return dve_ops._ANT_EXP_OPS
    from concourse.dve_spec import (Spec, Src0, C0, C1, C2, C3, One, maxx,
                                    minn, sq, _spill_c3_to_src1)
    from concourse.dve_ops import DveOp

    def expq_ref(in0, in1, c0, c1, c2):
        c3 = np.asarray(in1).reshape(np.asarray(in1).shape[0], -1)[:, :1]
        c3 = c3.astype(np.float32)
        up = np.maximum(in0.astype(np.float32), c0).astype(np.float32)
        u2 = (up * up).astype(np.float32)
        return ((c1 * up + c2).astype(np.float32) * u2
                + (c3 * up + 1.0)).astype(np.float32)

    _up = maxx(Src0, C0)
    _body = _spill_c3_to_src1((C1 * _up + C2) * sq(_up) + (C3 * _up + One))
    EXPQ = DveOp("EXPQ_POLY_ANT", Spec(body=_body, reference=expq_ref),
                 subdim=False, uops_sha={"v3": "251de3fd03c35767"})

    def pow64_ref(in0, in1, c0, c1, c2):
        y = in0.astype(np.float32)
        for _ in range(6):
            y = (y * y).astype(np.float32)
        return np.minimum(y, c0).astype(np.float32)

    _y = Src0
    for _ in range(6):
        _y = sq(_y)
    POW64 = DveOp("POW64_CLAMP_ANT",
                  Spec(body=minn(_y, C0), reference=pow64_ref),
                  subdim=False, uops_sha={"v3": "dfd7944c8736d43e"})

    for i, op in enumerate((EXPQ, POW64)):
        dve_ops.OPS.append(op)
        dve_ops.CUSTOM_DVE_SPECS[op.name] = op.spec
        dve_ops._SUB_OPCODE_FOR_NAME[op.name] = 17 + i
    dve_ops._ANT_EXP_OPS = (EXPQ, POW64)
    return dve_ops._ANT_EXP_OPS


def build_program():
    """Build the (shared, SPMD) Bass program. Returns compiled nc."""
    import concourse.bacc as bacc
    import concourse.mybir as mybir
    from concourse.tile import TileContext

    dt = mybir.dt
    f32 = dt.float32
    f32r = dt.float32r
    f8e4 = dt.float8e4
    f8e5 = dt.float8e5
    Sigmoid = mybir.ActivationFunctionType.Sigmoid
    mult = mybir.AluOpType.mult
    add = mybir.AluOpType.add
    DR = mybir.MatmulPerfMode.DoubleRow

    EXPQ, POW64 = _register_dve_ops()

    nc = bacc.Bacc("TRN2", target_bir_lowering=False, debug=False,
                   num_devices=NCORES)

    # --- I/O (PE operands declared f32r; host passes fp32 bits) ---
    # xw: host-packed, laid out identically to the SBUF tile so every DMA
    # is a dense contiguous copy: [wght(192) | x piece 0 (1024) |
    # wft(64), wot(256, rows 0-31), ones col + b-scalar(2), idm(32)
    # | x pieces 1-7 (7x1024)].
    xw = nc.dram_tensor("xw", [128, 1600 + 7 * 1024], f32r,
                        kind="ExternalInput").ap()
    aux = nc.dram_tensor("aux", [1, N], f32r, kind="ExternalInput").ap()
    res = nc.dram_tensor("res", [33, NH], f32, kind="ExternalOutput").ap()

    GC0 = 512
    NJT = N // 128            # 32 j-tiles
    NPR = NJT // 2            # 16 j-tile pairs for mm2
    NIG = NH // G             # 4 i-groups
    SUPERS = [2] * 16   # j-tiles per super (sum 32)
    assert sum(SUPERS) == NJT

    with TileContext(nc) as tc:
        with (
            tc.tile_pool(name="big", bufs=1) as bpool,
            tc.tile_pool(name="small", bufs=2) as spool,
            tc.tile_pool(name="qq", bufs=2) as qpool,
            tc.tile_pool(name="psA", bufs=1, space="PSUM") as psA,
            tc.tile_pool(name="psB", bufs=1, space="PSUM") as psB,
            tc.tile_pool(name="psC", bufs=1, space="PSUM") as psC,
            tc.tile_pool(name="pso", bufs=1, space="PSUM") as pso,
            tc.tile_pool(name="pst", bufs=1, space="PSUM") as pst,
        ):
            # --- weights + x in one identity-layout tile ---
            xall = bpool.tile([128, 1600 + 7 * 1024], f32r)
            wght_t = xall[:, 0:192]
            wft_t = xall[:, 1216:1280]
            ones_t = xall[0:32, 1536:1538]   # col0 zeros, col1 ones
            sca_f = xall[32:34, 1538:1539]   # [0, 64a]
            scb_f = xall[32:34, 1539:1540]   # [1/64, b_tot/64]
            idm_t = xall[0:32, 1568:1600]
            idm64_t = xall[64:96, 1568:1600]
            g_aug0 = bpool.tile([128, N], f32r, name="g_aug")
            nc.sync.dma_start(xall[:, 0:1600], xw[:, 0:1600])

            def emit_piece(gp):
                s0 = 1600 + (gp - 1) * 1024
                nc.sync.dma_start(xall[:, s0:s0 + 1024], xw[:, s0:s0 + 1024])

            for gp in (1, 2):
                emit_piece(gp)

            def xv(c, col, w):
                # x chunk c, columns [col, col+w) in piece-major layout
                gp = col // G
                assert col % G + w <= G
                base = 192 if gp == 0 else 1600 + (gp - 1) * 1024
                return xall[:, base + c * G + col % G:
                            base + c * G + col % G + w]

            # --- activation buffers ---
            f_t = [bpool.tile([128, G], f32r, name=f"f{gi}")
                   for gi in range(NIG)]
            g_aug = g_aug0    # rows: g(32), gbv(1), -1(1), h(32)@34
            hpt = bpool.tile([128, NJT * 64], f8e4)  # [h^T | 1 | 0pad] per j-tile
            hpt_v = hpt[:].rearrange("p (t w) -> p t w", w=64)
            nc.gpsimd.memset(hpt_v[:, :, 32:33], 1.0)   # den ones rows
            nc.gpsimd.memset(hpt_v[:, :, 33:64], 0.0)   # DR pad (M must be 32/64)
            eb_t = [bpool.tile([128, NJT * G], f8e5, name=f"eb{i}")
                    for i in range(2)]
            c3t = bpool.tile([128, 1], f32)
            nc.vector.memset(c3t[:], PC1)

            SPOOLS = (psA, psB, psC)
            SNAMES = ("sa", "sb", "sc")

            # --- stacked gh conv: psum rows [g(32); gbv(1); pad; h@64] ---
            GC = 512

            gh_pend = []

            def emit_gh_conv(grp):
                cps = pst.tile([128, GC], f32, name="tl")
                for c in range(2):
                    nc.tensor.matmul(
                        cps[0:96, :],
                        wght_t[:, c * 96:(c + 1) * 96],
                        xv(c, grp * GC, GC),
                        start=(c == 0), stop=(c == 1))
                sl = slice(grp * GC, (grp + 1) * GC)
                # one copy: rows [g(32); gbv(1); zeros; h(32)@64]; the
                # clobbered -1 row is restored by a tiny aux DMA
                nc.vector.tensor_copy(g_aug[0:96, sl], cps[0:96, :])
                nc.sync.dma_start(g_aug[33:34, sl], aux[:, sl])
                if grp + 3 <= 7:
                    emit_piece(grp + 3)
                gh_pend.append(grp)

            def emit_gh_flush():
                while gh_pend:
                    emit_gh_tps(gh_pend.pop(0))

            # transpose a group's 4 h j-tiles into hpt (fp8e4)
            def emit_gh_tps(grp):
                tps = pst.tile([128, 128], f32r, name="tlt", tag="tl")
                for k in range(4):
                    t = 4 * grp + k
                    nc.tensor.transpose(
                        tps[:, k * 32:(k + 1) * 32],
                        g_aug[64:96, t * 128:(t + 1) * 128],
                        idm64_t)
                nc.vector.tensor_copy(
                    hpt_v[:, 4 * grp:4 * grp + 4, 0:32],
                    tps[:].rearrange("p (t w) -> p t w", w=32).bitcast(f32))

            # --- f conv (own query half): rows [f/64; 1/64; m~/64] ---
            def emit_f_conv(fg, pool=None, pname="tl", sqpool=None,
                            sqname="tls"):
                cps = (pool or pst).tile([128, G], f32, name=pname)
                for c in range(2):
                    nc.tensor.matmul(
                        cps[0:32, :],
                        wft_t[:, c * 32:(c + 1) * 32],
                        xv(c, fg * G, G),
                        start=(c == 0), stop=(c == 1))
                nc.vector.tensor_copy(f_t[fg][0:32, :], cps[0:32, :])
                # rows 32:34 = [1/64 ; m~/64]: ssq' = sum_c (f_c/64)^2 via
                # elementwise square + PE ones-reduce into partitions 32:34
                # (row 32 of the reduce is zeroed by the zero weight col),
                # then per-partition affine [0,64a]*in + [1/64, b_tot/64]
                fsq = spool.tile([128, G], f32r, name="fsq")
                if fg < 2:
                    # startup critical path: DVE is idle; one PSUM input is
                    # allowed, the other reads the already-copied f tile
                    nc.vector.tensor_tensor(
                        fsq[0:32, :], cps[0:32, :], f_t[fg][0:32, :], mult)
                else:
                    nc.gpsimd.tensor_tensor(
                        fsq[0:32, :], f_t[fg][0:32, :], f_t[fg][0:32, :], mult)
                sq = (sqpool or pst).tile([128, G], f32, name=sqname,
                                           tag=None if sqpool else "tl")
                nc.tensor.matmul(sq[0:2, :], ones_t, fsq[0:32, :],
                                 start=True, stop=True)
                nc.vector.tensor_scalar(
                    f_t[fg][32:34, :], sq[0:2, :],
                    sca_f.bitcast(f32), scb_f.bitcast(f32), mult, add)

            # --- main attention loop (software-pipelined) ---
            stages = []
            for g in range(NIG):
                jt = 0
                for si, nt in enumerate(SUPERS):
                    stages.append((g, si, jt, nt))
                    jt += nt
            NS = len(stages)

            po_t = {}
            sps_t = {}
            npair = {g: 0 for g in range(NIG)}

            def emit_mm1(idx):
                g, si, jt, nt = stages[idx]
                pi = idx % 3
                sps = SPOOLS[pi].tile([128, nt * G], f32, name=SNAMES[pi])
                sps_t[idx] = sps
                for t in range(nt):
                    nc.tensor.matmul(
                        sps[:, t * G:(t + 1) * G],
                        g_aug[0:34, (jt + t) * 128:(jt + t + 1) * 128],
                        f_t[g][0:34, :],
                        start=True, stop=True)

            import os
            KN_DK = int(os.environ.get("KN_DK", "14"))
            KN_DEV = int(os.environ.get("KN_DEV", "4"))
            KN_DNU = int(os.environ.get("KN_DNU", "1"))

            def dve_stage(idx):
                return idx >= KN_DK and ((idx - KN_DK) % KN_DEV) < KN_DNU

            def emit_exp(idx):
                g, si, jt, nt = stages[idx]
                eb = eb_t[g % 2]
                if dve_stage(idx):
                    q = qpool.tile([128, 3 * G], f32, name="qq")
                    nc.vector._custom_dve(
                        EXPQ, out=q[:, 0:nt * G], in0=sps_t[idx][:, 0:nt * G],
                        in1=c3t[:], s0=CLO, s1=PC3, imm2=PC2)
                    nc.vector._custom_dve(
                        POW64, out=eb[:, jt * G:(jt + nt) * G],
                        in0=q[:, 0:nt * G], s0=1.0, s1=0.0, imm2=0.0)
                else:
                    nc.scalar.activation(
                        eb[:, jt * G:(jt + nt) * G], sps_t[idx][:, 0:nt * G],
                        Sigmoid, scale=64.0)

            def emit_mm2(idx, tlim=None):
                g, si, jt, nt = stages[idx]
                sps_t.pop(idx, None)
                if tlim is None:
                    tlim = jt + nt
                if npair[g] == 0 and tlim >= 2:
                    po_t[g] = pso.tile([128, G], f32, name="o")
                ebv = eb_t[g % 2][:].rearrange("p (t w) -> p t w", w=G)
                while (npair[g] + 1) * 2 <= tlim:
                    p = npair[g]
                    npair[g] += 1
                    nc.tensor.matmul(
                        po_t[g][0:64, :],
                        hpt_v[:, 2 * p:2 * p + 2, 0:64],
                        ebv[:, 2 * p:2 * p + 2, :],
                        start=(p == 0), stop=(p == NPR - 1),
                        perf_mode=DR)

            def emit_tail(g):
                po = po_t.pop(g)
                att = spool.tile([128, G], f32, name="att")
                nc.vector.tensor_copy(att[0:33, :], po[0:33, :])
                nc.sync.dma_start(res[:, g * G:(g + 1) * G], att[0:33, :])

            # Pipeline: mm1[k+1] issues before mm2[k]; gh-conv groups
            # trickle in between igrp-0 stages (DMA-gated anyway).
            import os
            KN_FSI = int(os.environ.get("KN_FSI", "0"))
            KN_CAH = int(os.environ.get("KN_CAH", "10"))
            convs_left = list(range(1, 8))
            f_left = list(range(1, NIG))
            proc = [0]
            emit_f_conv(0, pool=psB, pname="sb", sqpool=psC, sqname="sc")
            emit_gh_conv(0)
            emit_mm1(0)
            # mm2 for stage k-1 (its exp has had a full stage to finish, so
            # the PE queue head never blocks on ACT)
            for k in range(NS):
                emit_exp(k)
                g, si, jt, nt = stages[k]
                if k + 1 < NS:
                    # convs whose tiles mm1(k+1) reads MUST be emitted first
                    # (no writer => no dependency => stale reads)
                    njtn = stages[k + 1][2] + stages[k + 1][3]
                    while convs_left and convs_left[0] <= (njtn - 1) // 4:
                        emit_gh_conv(convs_left.pop(0))
                    emit_mm1(k + 1)
                if g == 0:
                    need = min((jt + nt + KN_CAH) // 4, 7)
                    while convs_left and convs_left[0] <= need:
                        emit_gh_conv(convs_left.pop(0))
                if f_left and si >= KN_FSI and f_left[0] <= g + 1:
                    emit_f_conv(f_left.pop(0))
                emit_gh_flush()
                # finalize stages whose exp has had >=2 stages to complete
                # (3 for DVE-chain stages: 2 serial DVE ops)
                while proc[0] < NS and k >= proc[0] + (3 if dve_stage(proc[0])
                                                       else 2):
                    pg, psi, pjt, pnt = stages[proc[0]]
                    emit_mm2(proc[0], tlim=pjt + pnt)
                    if npair[pg] == NPR and pg in po_t:
                        emit_tail(pg)
                    proc[0] += 1
            while proc[0] < NS:
                pg, psi, pjt, pnt = stages[proc[0]]
                emit_mm2(proc[0], tlim=pjt + pnt)
                if npair[pg] == NPR and pg in po_t:
                    emit_tail(pg)
                proc[0] += 1
            while convs_left:
                emit_gh_conv(convs_left.pop(0))
            emit_gh_flush()
            while f_left:
                emit_f_conv(f_left.pop(0))

    nc.compile()
    return nc


def _host_prep(Wv, bv, Wk, bk, Wq, bq, Wo, bo, gamma):
    gam = float(np.asarray(gamma).reshape(-1)[0])

    # stacked gh conv weights: rows = [Wk(32); bv@Wk(1); pad(31); Wq(32)]
    w_gh = np.zeros((96, 256), np.float32)
    w_gh[0:32] = Wk
    w_gh[32] = bv @ Wk
    w_gh[64:96] = Wq
    wght = np.zeros((128, 192), np.float32)
    for c in range(2):
        wght[:, c * 96:(c + 1) * 96] = w_gh.T[c * 128:(c + 1) * 128, :]

    # f conv: Wv scaled by 1/64 (keeps PSUM scores in sigmoid range)
    wft = np.zeros((128, 64), np.float32)
    for c in range(2):
        wft[:, c * 32:(c + 1) * 32] = (Wv.T / 64.0)[c * 128:(c + 1) * 128, :]

    # b_total for the m~ row: proto fit is against s that includes bv@bk;
    # the device's s excludes it, so fold it out here.
    b_tot = MB + MDELTA - float(bv @ bk)

    wpk = np.zeros((128, 1600), np.float32)
    wpk[:, 0:192] = wght
    wpk[:, 1216:1280] = wft
    wpk[0:32, 1537] = 1.0                      # ones col (ssq reduce)
    wpk[33, 1538] = 64.0 * MA                  # m~ affine scale
    wpk[32, 1539] = 1.0 / 64.0                 # f row 32 const
    wpk[33, 1539] = b_tot / 64.0               # m~ affine bias
    wpk[0:32, 1568:1600] = np.eye(32)          # idm
    wpk[64:96, 1568:1600] = np.eye(32)         # idm at base 64 (h transposes)
    return wpk


def kernel(**inputs):
    from concourse.bass_utils import run_bass_kernel_spmd

    x = np.asarray(inputs["x"], np.float32)
    Wo = np.asarray(inputs["Wo"], np.float32)
    bq = np.asarray(inputs["bq"], np.float32)
    bo = np.asarray(inputs["bo"], np.float32)
    gam = float(np.asarray(inputs["gamma"]).reshape(-1)[0])
    consts = _host_prep(
        np.asarray(inputs["Wv"], np.float32),
        np.asarray(inputs["bv"], np.float32),
        np.asarray(inputs["Wk"], np.float32),
        np.asarray(inputs["bk"], np.float32),
        np.asarray(inputs["Wq"], np.float32),
        bq, Wo, bo, gam,
    )

    if "nc" not in _CACHE:
        _CACHE["nc"] = build_program()
    nc = _CACHE["nc"]

    in_maps = []
    for core in range(NCORES):
        b, ih = core // 2, core % 2
        # rotate keys so this core's query half sits at columns 0..NH-1
        # (softmax is invariant to key order, so this is exact), then pack
        # [weights | x] in the kernel's piece-major SBUF layout
        xrot = np.roll(x[b], -ih * NH, axis=1)
        xp = (xrot.reshape(2, 128, 8, 512)
              .transpose(1, 2, 0, 3).reshape(128, 8, 1024))
        xw = np.empty((128, 1600 + 7 * 1024), np.float32)
        xw[:, 0:1600] = consts
        xw[:, 192:1216] = xp[:, 0]
        xw[:, 1600:] = xp[:, 1:].reshape(128, 7 * 1024)
        in_maps.append({"xw": xw, "aux": np.full((1, N), -1.0, np.float32)})

    r = run_bass_kernel_spmd(nc, in_maps, core_ids=list(range(NCORES)),
                             trace=False)
    bof = (gam * (Wo @ bq + bo)).astype(np.float32)   # [256]
    gWo = (gam * Wo).astype(np.float32)
    out = np.empty((B, C, N), np.float32)
    for core in range(NCORES):
        b, ih = core // 2, core % 2
        R = r.results[core]["res"]                    # [33, NH]: num; den
        d = np.where(R[32] == 0.0, 1.0, R[32])
        sl = slice(ih * NH, (ih + 1) * NH)
        out[b][:, sl] = gWo @ (R[0:32] / d[None, :]) + bof[:, None] + x[b][:, sl]
    return out


if __name__ == "__main__":
    nc = build_program()
    print("program built ok")
